# revision 11
# baseline (speedup 1.0000x reference)
"""MoE FFN (top-2, capacity-dropped) on 8 trn2 NeuronCores, expert-parallel.

Strategy:
  - Host computes routing (gating matmul, top-2, capacity cumsum, renorm).
  - Each core owns one expert: gathers its tokens via indirect DMA,
    transposes to xeT, runs the two FFN GEMMs (float32r full-rate matmuls),
    scales rows by combine weights, scatters into a [T,D] partial-y buffer,
    then an 8-way ReduceScatter yields each core's token shard of y.
  - aux_loss is a cheap scalar computed on host from routing stats.
"""

import numpy as np

import concourse.bacc as bacc
import concourse.tile as tile
from concourse import bass, mybir
from concourse.bass_utils import run_bass_kernel_spmd
from concourse.kernels.tile_matmul import matmul_tile_kernel
from concourse.masks import make_identity

P = 128
B, S, D, F, E = 4, 2048, 1024, 4096, 8
T = B * S                    # 8192 tokens
C = 1280                     # capacity = ceil(1.25 * T / E)
NCORE = 8
TSH = T // NCORE             # 1024 tokens per output shard
CT = C // P                  # 10 capacity tiles
DUMMY = T                    # scatter destination for invalid slots

F32 = mybir.dt.float32
F32R = mybir.dt.float32r
I32 = mybir.dt.int32

_prog = None
_last_results = None


def _build(stages=("gather", "gemm", "scatter", "zero", "rs")):
    nc = bacc.Bacc("TRN2", target_bir_lowering=False, debug=False, num_devices=NCORE)
    x = nc.dram_tensor("x", [T, D], F32, kind="ExternalInput").ap()
    w1 = nc.dram_tensor("w1", [D, F], F32R, kind="ExternalInput").ap()
    w2 = nc.dram_tensor("w2", [F, D], F32R, kind="ExternalInput").ap()
    mtok = nc.dram_tensor("mtok", [C, 1], I32, kind="ExternalInput").ap()
    mdst = nc.dram_tensor("mdst", [C, 1], I32, kind="ExternalInput").ap()
    mw = nc.dram_tensor("mw", [C, 1], F32, kind="ExternalInput").ap()
    ysh = nc.dram_tensor("ysh", [TSH, D], F32, kind="ExternalOutput").ap()

    xeT = nc.dram_tensor("xeT", [D, C], F32, kind="Internal").ap()
    hT = nc.dram_tensor("hT", [F, C], F32, kind="Internal").ap()
    ye = nc.dram_tensor("ye", [C, D], F32, kind="Internal").ap()
    ypart = nc.dram_tensor("ypart", [T + P, D], F32, kind="Internal").ap()
    yshb = nc.dram_tensor("yshb", [TSH, D], F32, kind="Internal").ap()

    with tile.TileContext(nc) as tc:
        with (
            tc.tile_pool(name="gio", bufs=3) as gio,
            tc.tile_pool(name="gtr", bufs=3) as gtr,
            tc.tile_pool(name="gps", bufs=2, space="PSUM") as gps,
            tc.tile_pool(name="small", bufs=4) as small,
            tc.tile_pool(name="const", bufs=1) as const,
        ):
            ident = const.tile([P, P], F32)
            make_identity(nc, ident[:])

            # zero the partial-y buffer (incl. the dummy tile at rows T..T+P)
            zt = const.tile([P, D], F32)
            nc.vector.memset(zt[:], 0.0)
            for i in range((T + P) // P if "zero" in stages else 0):
                nc.sync.dma_start(ypart[i * P : (i + 1) * P, :], zt[:])

            # gather this expert's tokens and store transposed xeT [D, C]
            xeT_r = xeT.rearrange("(db p) c -> p db c", p=P)
            for i in range(CT if "gather" in stages else 0):
                it = small.tile([P, 1], I32)
                nc.sync.dma_start(it[:], mtok[i * P : (i + 1) * P, :])
                xt = gio.tile([P, D], F32)
                nc.gpsimd.indirect_dma_start(
                    out=xt[:],
                    out_offset=None,
                    in_=x,
                    in_offset=bass.IndirectOffsetOnAxis(ap=it[:, :1], axis=0),
                )
                xtT = gtr.tile([P, D // P, P], F32)
                for db in range(D // P):
                    pst = gps.tile([P, P], F32)
                    nc.tensor.transpose(pst[:], xt[:, db * P : (db + 1) * P], ident[:])
                    nc.vector.tensor_copy(xtT[:, db, :], pst[:])
                nc.sync.dma_start(xeT_r[:, :, i * P : (i + 1) * P], xtT[:])

            if "gemm" in stages:
                # hT[F, C] = relu(w1.T @ xeT)  (kxm = w1 [K=D, M=F], kxn = xeT [K=D, N=C])
                matmul_tile_kernel(tc, kxm_ap=w1, kxn_ap=xeT, mxn_ap=hT, use_relu=True, matmul_dtype=F32R)
                # ye[C, D] = hT.T @ w2         (kxm = hT [K=F, M=C], kxn = w2 [K=F, N=D])
                matmul_tile_kernel(tc, kxm_ap=hT, kxn_ap=w2, mxn_ap=ye, matmul_dtype=F32R)

            # scale rows by combine weight, scatter into ypart rows (token ids)
            for i in range(CT if "scatter" in stages else 0):
                yt = gio.tile([P, D], F32)
                nc.sync.dma_start(yt[:], ye[i * P : (i + 1) * P, :])
                wt = small.tile([P, 1], F32)
                nc.sync.dma_start(wt[:], mw[i * P : (i + 1) * P, :])
                dt_ = small.tile([P, 1], I32)
                nc.sync.dma_start(dt_[:], mdst[i * P : (i + 1) * P, :])
                nc.vector.tensor_scalar_mul(yt[:], yt[:], wt[:, :1])
                nc.gpsimd.indirect_dma_start(
                    out=ypart,
                    out_offset=bass.IndirectOffsetOnAxis(ap=dt_[:, :1], axis=0),
                    in_=yt[:],
                    in_offset=None,
                )

            if "rs" in stages:
                nc.gpsimd.collective_compute(
                    "ReduceScatter",
                    mybir.AluOpType.add,
                    replica_groups=[list(range(NCORE))],
                    ins=[ypart[0:T, :]],
                    outs=[yshb],
                )
                nc.sync.dma_start(ysh, yshb)
            else:
                nc.sync.dma_start(ysh, ypart[0:TSH, :])

    nc.compile()
    return nc


def _build_v2():
    """A2A-combine design: GEMM2 writes compact ye; AllGather all experts'
    ye; each core gathers+weights the two expert rows for its token shard."""
    nc = bacc.Bacc("TRN2", target_bir_lowering=False, debug=False, num_devices=NCORE)
    x = nc.dram_tensor("x", [T, D], F32, kind="ExternalInput").ap()
    w1 = nc.dram_tensor("w1", [D, F], F32R, kind="ExternalInput").ap()
    w2 = nc.dram_tensor("w2", [F, D], F32R, kind="ExternalInput").ap()
    mtok = nc.dram_tensor("mtok", [C, 1], I32, kind="ExternalInput").ap()
    cidx = nc.dram_tensor("cidx", [2 * TSH, 1], I32, kind="ExternalInput").ap()
    cw = nc.dram_tensor("cw", [2 * TSH, 1], F32, kind="ExternalInput").ap()
    ysh = nc.dram_tensor("ysh", [TSH, D], F32, kind="ExternalOutput").ap()

    xeT = nc.dram_tensor("xeT", [D, C], F32, kind="Internal").ap()
    hT = nc.dram_tensor("hT", [F, C], F32, kind="Internal").ap()
    ye = nc.dram_tensor("ye", [C, D], F32, kind="Internal").ap()
    yeall = nc.dram_tensor(
        "yeall", [NCORE * C, D], F32, kind="Internal", addr_space="Shared"
    ).ap()

    with tile.TileContext(nc) as tc:
        with (
            tc.tile_pool(name="gio", bufs=4) as gio,
            tc.tile_pool(name="gtr", bufs=3) as gtr,
            tc.tile_pool(name="gps", bufs=2, space="PSUM") as gps,
            tc.tile_pool(name="small", bufs=4) as small,
            tc.tile_pool(name="const", bufs=1) as const,
        ):
            ident = const.tile([P, P], F32)
            make_identity(nc, ident[:])

            xeT_r = xeT.rearrange("(db p) c -> p db c", p=P)
            for i in range(CT):
                it = small.tile([P, 1], I32)
                nc.sync.dma_start(it[:], mtok[i * P : (i + 1) * P, :])
                xt = gio.tile([P, D], F32, tag="io")
                nc.gpsimd.indirect_dma_start(
                    out=xt[:],
                    out_offset=None,
                    in_=x,
                    in_offset=bass.IndirectOffsetOnAxis(ap=it[:, :1], axis=0),
                )
                xtT = gtr.tile([P, D // P, P], F32, tag="tr")
                for db in range(D // P):
                    pst = gps.tile([P, P], F32)
                    nc.tensor.transpose(pst[:], xt[:, db * P : (db + 1) * P], ident[:])
                    nc.vector.tensor_copy(xtT[:, db, :], pst[:])
                nc.sync.dma_start(xeT_r[:, :, i * P : (i + 1) * P], xtT[:])

            matmul_tile_kernel(tc, kxm_ap=w1, kxn_ap=xeT, mxn_ap=hT,
                               use_relu=True, matmul_dtype=F32R)
            matmul_tile_kernel(tc, kxm_ap=hT, kxn_ap=w2, mxn_ap=ye,
                               matmul_dtype=F32R)

            nc.gpsimd.collective_compute(
                "AllGather",
                mybir.AluOpType.bypass,
                replica_groups=[list(range(NCORE))],
                ins=[ye],
                outs=[yeall],
            )

            for st in range(TSH // P):
                i0 = small.tile([P, 1], I32)
                nc.sync.dma_start(i0[:], cidx[st * P : (st + 1) * P, :])
                i1 = small.tile([P, 1], I32)
                nc.sync.dma_start(i1[:], cidx[TSH + st * P : TSH + (st + 1) * P, :])
                w0 = small.tile([P, 1], F32)
                nc.sync.dma_start(w0[:], cw[st * P : (st + 1) * P, :])
                w1t = small.tile([P, 1], F32)
                nc.sync.dma_start(w1t[:], cw[TSH + st * P : TSH + (st + 1) * P, :])
                g0 = gio.tile([P, D], F32, tag="io")
                nc.gpsimd.indirect_dma_start(
                    out=g0[:], out_offset=None, in_=yeall,
                    in_offset=bass.IndirectOffsetOnAxis(ap=i0[:, :1], axis=0),
                )
                g1 = gio.tile([P, D], F32, tag="io")
                nc.gpsimd.indirect_dma_start(
                    out=g1[:], out_offset=None, in_=yeall,
                    in_offset=bass.IndirectOffsetOnAxis(ap=i1[:, :1], axis=0),
                )
                nc.vector.tensor_scalar_mul(g0[:], g0[:], w0[:, :1])
                nc.vector.tensor_scalar_mul(g1[:], g1[:], w1t[:, :1])
                yo = gtr.tile([P, D], F32, tag="tr")
                nc.vector.tensor_add(yo[:], g0[:], g1[:])
                nc.sync.dma_start(ysh[st * P : (st + 1) * P, :], yo[:])

    nc.compile()
    return nc


def _route(xf, gw):
    """Replicates the reference routing exactly (fp32 numpy)."""
    logits = xf @ gw                                    # [T, E]
    order = np.argsort(-logits, axis=1, kind="stable")[:, :2]
    v0 = logits[np.arange(T), order[:, 0]]
    v1 = logits[np.arange(T), order[:, 1]]
    z = np.exp(v1 - v0)
    p0 = 1.0 / (1.0 + z)
    p1 = z / (1.0 + z)

    flat_idx = order.reshape(-1)                        # [T*2] token-major
    pos = np.empty(T * 2, np.int64)
    for ee in range(E):
        m = flat_idx == ee
        pos[m] = np.arange(m.sum())
    keep = pos < C

    pr = (np.stack([p0, p1], 1).reshape(-1) * keep).reshape(T, 2).astype(np.float32)
    w = (pr / (pr.sum(1, keepdims=True) + np.float32(1e-9))).reshape(-1)

    tok = np.repeat(np.arange(T), 2)
    mtokA = np.zeros((E, C, 1), np.int32)
    mdstA = np.full((E, C, 1), DUMMY, np.int32)
    mwA = np.zeros((E, C, 1), np.float32)
    for ee in range(E):
        m = (flat_idx == ee) & keep
        ss = pos[m]
        mtokA[ee, ss, 0] = tok[m]
        mdstA[ee, ss, 0] = tok[m]
        mwA[ee, ss, 0] = w[m]

    routed = (w > 0).astype(np.float64)
    counts = np.bincount(flat_idx, weights=routed, minlength=E)
    importance = np.bincount(flat_idx, weights=w.astype(np.float64), minlength=E)
    tf = counts / (counts.sum() + 1e-9)
    imf = importance / (importance.sum() + 1e-9)
    aux = np.float32((tf * imf).sum() * E)

    # combine metadata: per token-slot, row in the AllGathered [E*C, D] buffer
    gidx = (flat_idx * C + np.minimum(pos, C - 1)).astype(np.int32).reshape(T, 2)
    wTK = w.reshape(T, 2)
    cidxA = np.zeros((NCORE, 2 * TSH, 1), np.int32)
    cwA = np.zeros((NCORE, 2 * TSH, 1), np.float32)
    for cc in range(NCORE):
        sl = slice(cc * TSH, (cc + 1) * TSH)
        cidxA[cc, :TSH, 0] = gidx[sl, 0]
        cidxA[cc, TSH:, 0] = gidx[sl, 1]
        cwA[cc, :TSH, 0] = wTK[sl, 0]
        cwA[cc, TSH:, 0] = wTK[sl, 1]
    return mtokA, mdstA, mwA, aux, cidxA, cwA


def kernel(x, gate_w, w1, w2, k, _trace=False):
    global _prog, _last_results
    assert int(k) == 2
    x = np.ascontiguousarray(np.asarray(x, np.float32).reshape(T, D))
    gw = np.ascontiguousarray(np.asarray(gate_w, np.float32))
    w1 = np.ascontiguousarray(np.asarray(w1, np.float32))
    w2 = np.ascontiguousarray(np.asarray(w2, np.float32))

    mtokA, mdstA, mwA, aux, cidxA, cwA = _route(x, gw)

    if _prog is None:
        _prog = _build_v2()

    in_maps = [
        {
            "x": x,
            "w1": w1[c],
            "w2": w2[c],
            "mtok": mtokA[c],
            "cidx": cidxA[c],
            "cw": cwA[c],
        }
        for c in range(NCORE)
    ]
    res = run_bass_kernel_spmd(
        _prog, in_maps, core_ids=list(range(NCORE)), trace=_trace
    )
    _last_results = res
    y = np.concatenate([res.results[c]["ysh"] for c in range(NCORE)], axis=0)
    return y.reshape(B, S, D), aux
